# revision 23
# baseline (speedup 1.0000x reference)
"""Trainium2 Bass kernel for nn_Decoder (sparse_attention).

Reference computation (per batch b):
  knn   = top-3 stations by l[b]                         (sparse attention support)
  q_in  = sum_n l[b,n] * H[b,t,n,:]                      [T,F]
  q     = q_in @ Wq.T + bq
  keys  = H @ Wk.T + bk   (only needed at knn stations)
  attn  = softmax over the 3 knn stations of q . keys
  vals  = H @ Wv.T + bv   (only needed at knn stations)
  h_kn  = sum_k attn_k * vals_k = Wv @ (sum_k attn_k * Hsel_k) + bv
  h     = relu(concat([q_in, h_kn]) @ Wkk.T + bkk)
  x     = GRU_2layer(h); out = relu(x[:,-1,:] @ Wo.T + bo)

Kernel strategy (8 cores, data-parallel over batch, 8 batches/core):
  Phase 1: stream H[b] tiles [n=128, t*F] through the PE as the stationary
    operand against a small selection matrix S_b [128, 4] whose columns are
    (l[b], onehot(knn0), onehot(knn1), onehot(knn2)).  One pass over H
    produces both q_in and the 3 gathered stations with F on partitions.
  Phase 2: per quarter-batch (96 (b,t) columns), pipelined under the
    phase-1 DMA of later quarters: q/keys projections, scores via
    elementwise-mul + ones-matmul partition reduction, 3-way softmax,
    attn broadcast via ones-matmul, station mix, Wv and Wkk projections,
    relu, and the layer-1 bulk gi.  All projection matmuls run in fp16
    (1 cycle/row vs fp32's 4).
  Phase 3: 2-layer GRU, software-pipelined: layer 1's step is split
    across the round boundary (head at round end, mid+tail at next round
    start) so the in-order ACT queue matches readiness order.  Per step,
    three PSUM banks Pr/Pn/Pz (r-gate inject+matmuls first) let sig_r
    fire after 5 matmuls; gi_n is added from SBUF, sig_z retires off the
    critical path.  Steady round (both layers) ~2.37us on HW.

Precision: H and S stream as fp8 e4m3 (~12.6 MiB/core, halves the fp16
HBM roofline); everything else fp16 with fp32 PSUM accumulation.  The
final ReLU output margin is ~0.013 vs ~2.4e-3 total fp8-induced error
(verified via BASS_DEC_RAW=1).  Env BASS_DEC_PREC=f8|f16|f32 (default f8).
"""

import os
import sys
from contextlib import ExitStack

import numpy as np

for _p in ("/opt/trn_rl_repo", "/root/.axon_site/_ro/trn_rl_repo"):
    if os.path.isdir(_p) and _p not in sys.path:
        sys.path.insert(0, _p)

B, T, N, F, L = 64, 48, 128, 256, 2
NCORES = 8
BL = B // NCORES          # local batch per core
BT = BL * T               # phase-2 column count
TC = 24                   # t-chunk for phase-1 DMA/matmul
G = 6                     # gate row-slices (3F/128)

_PREC = os.environ.get("BASS_DEC_PREC", "f8")
_NC_CACHE = {}


def _np_dt(prec):
    return np.float32 if prec == "f32" else np.float16


def _np_stream_dt(prec):
    if prec == "f8":
        import ml_dtypes
        return ml_dtypes.float8_e4m3fn
    return _np_dt(prec)


def _build(zero_bias, prec):
    from concourse import bacc, tile, mybir

    dt = mybir.dt
    f32 = dt.float32
    dth = dt.float32 if prec == "f32" else dt.float16
    # stream dtype for the big H tensor (and the selection matrix S it is
    # multiplied with): fp8 e4m3 halves the HBM roofline again vs fp16
    dts = dt.float8e4 if prec == "f8" else dth

    AF = mybir.ActivationFunctionType
    OP = mybir.AluOpType

    nc = bacc.Bacc("TRN2", target_bir_lowering=False, debug=False,
                   num_devices=NCORES)

    # ---- DRAM I/O (per-core shard) ----
    Hd = nc.dram_tensor("H", [BL, N, T, F], dts, kind="ExternalInput")
    Sd = nc.dram_tensor("S", [N, BL, 4], dts, kind="ExternalInput")
    Wqd = nc.dram_tensor("WqT", [128, 2, F], dth, kind="ExternalInput")
    Wkd = nc.dram_tensor("WkT", [128, 2, F], dth, kind="ExternalInput")
    Wvd = nc.dram_tensor("WvT", [128, 2, F], dth, kind="ExternalInput")
    Wkkd = nc.dram_tensor("WkkT", [128, 4, F], dth, kind="ExternalInput")
    Wihd = [nc.dram_tensor(f"WihT{i}", [128, 2, 3 * F], dth,
                           kind="ExternalInput") for i in range(L)]
    Whhd = [nc.dram_tensor(f"WhhT{i}", [128, 2, 3 * F], dth,
                           kind="ExternalInput") for i in range(L)]
    Wod = nc.dram_tensor("WoT", [128, 2, 1], dth, kind="ExternalInput")
    bqd = nc.dram_tensor("bq", [128, 2], f32, kind="ExternalInput")
    bkd = nc.dram_tensor("bk", [128, 2], f32, kind="ExternalInput")
    bvd = nc.dram_tensor("bv", [128, 2], f32, kind="ExternalInput")
    bkkd = nc.dram_tensor("bkk", [128, 2], f32, kind="ExternalInput")
    bihd = [nc.dram_tensor(f"bih{i}", [128, G], f32, kind="ExternalInput")
            for i in range(L)]
    bhhd = [nc.dram_tensor(f"bhh{i}", [128, G], f32, kind="ExternalInput")
            for i in range(L)]
    bod = nc.dram_tensor("bo", [BL, 1], f32, kind="ExternalInput")
    eyed = nc.dram_tensor("EYE", [128, 128], dth, kind="ExternalInput")
    outd = nc.dram_tensor("out", [BL, 1], f32, kind="ExternalOutput")

    with tile.TileContext(nc) as tc, ExitStack() as ctx:
        cpool = ctx.enter_context(tc.tile_pool(name="consts", bufs=1))
        persist = ctx.enter_context(tc.tile_pool(name="persist", bufs=1))

        # ---- load parameters to SBUF ----
        sS = cpool.tile([N, BL, 4], dts)
        nc.sync.dma_start(sS[:], Sd.ap()[:])
        wq = cpool.tile([128, 2, F], dth)
        nc.sync.dma_start(wq[:], Wqd.ap()[:])
        wk = cpool.tile([128, 2, F], dth)
        nc.sync.dma_start(wk[:], Wkd.ap()[:])
        wv = cpool.tile([128, 2, F], dth)
        nc.sync.dma_start(wv[:], Wvd.ap()[:])
        wkk = cpool.tile([128, 4, F], dth)
        nc.sync.dma_start(wkk[:], Wkkd.ap()[:])
        wih = []
        whh = []
        for i in range(L):
            wih_i = cpool.tile([128, 2, 3 * F], dth, name=f"wih{i}")
            wih.append(wih_i)
            whh_i = cpool.tile([128, 2, 3 * F], dth, name=f"whh{i}")
            whh.append(whh_i)
        # wih[0] feeds the phase-2 gi bulk: load it up front.  The rest of
        # the GRU-only weights are DMA'd after the phase emission (below)
        # so the H stream starts at the head of the DMA queues.
        nc.sync.dma_start(wih[0][:], Wihd[0].ap()[:])
        wo = cpool.tile([128, 2, 1], dth)
        bo_sb = cpool.tile([BL, 1], f32)
        if not zero_bias:
            bq_sb = cpool.tile([128, 2], f32)
            nc.sync.dma_start(bq_sb[:], bqd.ap()[:])
            bk_sb = cpool.tile([128, 2], f32)
            nc.sync.dma_start(bk_sb[:], bkd.ap()[:])
            bv_sb = cpool.tile([128, 2], f32)
            nc.sync.dma_start(bv_sb[:], bvd.ap()[:])
            bkk_sb = cpool.tile([128, 2], f32)
            nc.sync.dma_start(bkk_sb[:], bkkd.ap()[:])
            bih_sb = []
            bhh_sb = []
            for i in range(L):
                bih_i = cpool.tile([128, G], f32, name=f"bih_sb{i}")
                nc.sync.dma_start(bih_i[:], bihd[i].ap()[:])
                bih_sb.append(bih_i)
                bhh_i = cpool.tile([128, G], f32, name=f"bhh_sb{i}")
                nc.sync.dma_start(bhh_i[:], bhhd[i].ap()[:])
                bhh_sb.append(bhh_i)

        ones_col = cpool.tile([128, 1], f32)      # scores reduction lhsT
        nc.gpsimd.memset(ones_col[:], 1.0)
        ones_row = cpool.tile([1, 128], f32)      # broadcast lhsT
        nc.gpsimd.memset(ones_row[:], 1.0)
        eye = cpool.tile([128, 128], dth)         # identity: psum-inject lhsT

        # X[p, s, b, t, c]: c=0 -> q_in, c=1..3 -> selected stations
        # X split per QUARTER-batch so phase 2 of quarter q starts while
        # phase-1 DMA of quarter q+1 is still streaming (Tile deps are
        # whole-tile, not per-slice).  fp16: phase-2 matmuls then run at
        # 1 cycle/row instead of fp32's 4.
        NQ = 4
        QB = BL // NQ
        Xh = [persist.tile([128, 2, QB, T, 4], dth, name=f"X{q}")
              for q in range(NQ)]
        Xgru = persist.tile([128, 2, BL, T], dth)   # phase-2 output h
        # bulk gi for layer 1 (fp16 in the fast path: re-injected into
        # PSUM by an identity matmul each step)
        GIb = persist.tile([128, G, BL, T], dth if zero_bias else f32)
        Y1 = persist.tile([128, 2, BL, T], dth)
        Y2 = persist.tile([128, 2, BL, T], dth)

        # one shared PSUM pool for all phases: 8 rotating bank slots, so
        # phases pipeline instead of serializing on pool address reuse
        pp = ctx.enter_context(tc.tile_pool(name="pp", bufs=8, space="PSUM"))
        hp = ctx.enter_context(tc.tile_pool(name="hload", bufs=8))
        p2 = ctx.enter_context(tc.tile_pool(name="p2", bufs=1))
        gs = ctx.enter_context(tc.tile_pool(name="gs", bufs=3))

        # =========== Phase 1: q_in + knn gather (one pass over H) ==========
        def phase1(b):
            for tci in range(T // TC):
                ht = hp.tile([128, TC, F], dts, tag="ht", name="ht")
                nc.sync.dma_start(
                    ht[:], Hd.ap()[b, :, tci * TC:(tci + 1) * TC, :])
                pt = pp.tile([128, 2, TC, 4], f32, tag="bank", name="pt")
                for s in range(2):
                    for ti in range(TC):
                        nc.tensor.matmul(
                            pt[:, s, ti, :],
                            lhsT=ht[:, ti, s * 128:(s + 1) * 128],
                            rhs=sS[:, b, :],
                            start=True, stop=True)
                nc.vector.tensor_copy(
                    Xh[b // QB][:, :, b % QB, tci * TC:(tci + 1) * TC, :],
                    pt[:])

        # =========== Phase 2: attention + mix + mlp ========================
        # done per quarter-batch so it overlaps phase-1 DMA of later batches
        def phase2(p2, pp2, q):
            b0, b1 = q * QB, (q + 1) * QB
            nb = QB * T
            XH = Xh[q]
            rhs_qin = XH[:, :, :, :, 0]
            prodS = p2.tile([128, 3, 2, nb], f32, tag="prodS", bufs=2,
                            name=f"prodS{q}")
            pq = []
            for ms in range(2):
                pq_ms = pp2.tile([128, nb], f32, tag="bank",
                                 name=f"pq{q}{ms}")
                for ks in range(2):
                    nc.tensor.matmul(
                        pq_ms[:],
                        lhsT=wq[:, ks, ms * 128:(ms + 1) * 128],
                        rhs=rhs_qin[:, ks],
                        start=(ks == 0), stop=(ks == 1))
                pq.append(pq_ms)
            for k in range(3):
                for ms in range(2):
                    pk = pp2.tile([128, nb], f32, tag="bank",
                                  name=f"pk{q}{k}{ms}")
                    for ks in range(2):
                        nc.tensor.matmul(
                            pk[:],
                            lhsT=wk[:, ks, ms * 128:(ms + 1) * 128],
                            rhs=XH[:, ks, :, :, k + 1],
                            start=(ks == 0), stop=(ks == 1))
                    ksb = p2.tile([128, nb], f32, tag="ksb", bufs=2,
                                  name=f"ksb{q}{k}{ms}")
                    if zero_bias:
                        nc.vector.tensor_copy(ksb[:], pk[:])
                        nc.vector.tensor_tensor(
                            prodS[:, k, ms, :], ksb[:], pq[ms][:], OP.mult)
                    else:
                        nc.vector.tensor_scalar_add(
                            ksb[:], pk[:], bk_sb[:, ms:ms + 1])
                        nc.vector.scalar_tensor_tensor(
                            prodS[:, k, ms, :], pq[ms][:],
                            bq_sb[:, ms:ms + 1], ksb[:],
                            op0=OP.add, op1=OP.mult)
            psc = []
            for k in range(3):
                ps = pp2.tile([1, nb], f32, tag="bank", name=f"ps{q}{k}")
                for ms in range(2):
                    nc.tensor.matmul(
                        ps[:], lhsT=ones_col[:, 0:1], rhs=prodS[:, k, ms, :],
                        start=(ms == 0), stop=(ms == 1))
                psc.append(ps)
            E = p2.tile([1, 3, nb], f32, tag="E", bufs=2, name=f"E{q}")
            for k in range(3):
                nc.scalar.activation(E[:, k, :], psc[k][:], AF.Exp)
            s2 = p2.tile([1, nb], f32, tag="s2", bufs=2, name=f"s2_{q}")
            nc.vector.tensor_add(s2[:], E[:, 0, :], E[:, 1, :])
            ssum = p2.tile([1, nb], f32, tag="ssum", bufs=2, name=f"ssum{q}")
            nc.vector.tensor_add(ssum[:], s2[:], E[:, 2, :])
            rec = p2.tile([1, nb], f32, tag="rec", bufs=2, name=f"rec{q}")
            nc.vector.reciprocal(rec[:], ssum[:])
            attn = p2.tile([1, 3, nb], f32, tag="attn", bufs=2,
                           name=f"attn{q}")
            for k in range(3):
                nc.vector.tensor_tensor(
                    attn[:, k, :], E[:, k, :], rec[:], OP.mult)
            pb = []
            for k in range(3):
                pb_k = pp2.tile([128, nb], f32, tag="bank",
                                name=f"pb{q}{k}")
                nc.tensor.matmul(pb_k[:], lhsT=ones_row[0:1, :],
                                 rhs=attn[:, k, :], start=True, stop=True)
                pb.append(pb_k)
            hm = p2.tile([128, 2, nb], dth, tag="hm", bufs=2, name=f"hm{q}")
            for s in range(2):
                m0 = p2.tile([128, nb], f32, tag="mixt", bufs=2,
                             name=f"m0_{q}{s}")
                nc.vector.tensor_tensor(
                    m0[:], pb[0][:], XH[:, s, :, :, 1], OP.mult)
                m1 = p2.tile([128, nb], f32, tag="mixt", bufs=2,
                             name=f"m1_{q}{s}")
                nc.vector.tensor_tensor(
                    m1[:], pb[1][:], XH[:, s, :, :, 2], OP.mult)
                a0 = p2.tile([128, nb], f32, tag="mixa", bufs=2,
                             name=f"a0_{q}{s}")
                nc.vector.tensor_add(a0[:], m0[:], m1[:])
                m2 = p2.tile([128, nb], f32, tag="mixt", bufs=2,
                             name=f"m2_{q}{s}")
                nc.vector.tensor_tensor(
                    m2[:], pb[2][:], XH[:, s, :, :, 3], OP.mult)
                nc.vector.tensor_add(hm[:, s, :], a0[:], m2[:])
            vsb = p2.tile([128, 2, nb], dth, tag="vsb", bufs=2,
                          name=f"vsb{q}")
            for ms in range(2):
                pv = pp2.tile([128, nb], f32, tag="bank",
                              name=f"pv{q}{ms}")
                for ks in range(2):
                    nc.tensor.matmul(
                        pv[:], lhsT=wv[:, ks, ms * 128:(ms + 1) * 128],
                        rhs=hm[:, ks, :], start=(ks == 0), stop=(ks == 1))
                if zero_bias:
                    nc.vector.tensor_copy(vsb[:, ms, :], pv[:])
                else:
                    nc.vector.tensor_scalar_add(
                        vsb[:, ms, :], pv[:], bv_sb[:, ms:ms + 1])
            for ms in range(2):
                ph = pp2.tile([128, nb], f32, tag="bank",
                              name=f"ph{q}{ms}")
                for ks in range(4):
                    rhs = rhs_qin[:, ks] if ks < 2 else vsb[:, ks - 2, :]
                    nc.tensor.matmul(
                        ph[:], lhsT=wkk[:, ks, ms * 128:(ms + 1) * 128],
                        rhs=rhs, start=(ks == 0), stop=(ks == 3))
                bias = 0.0 if zero_bias else bkk_sb[:, ms:ms + 1]
                nc.scalar.activation(Xgru[:, ms, b0:b1, :], ph[:], AF.Relu,
                                     bias=bias)

        # layer-1 bulk gi for this quarter while DMA continues
        def phase2_gi(p2, pp2, q):
            b0, b1 = q * QB, (q + 1) * QB
            nb = QB * T
            for m in range(G):
                pg = pp2.tile([128, nb], f32, tag="bank",
                              name=f"pg{q}{m}")
                for ks in range(2):
                    nc.tensor.matmul(
                        pg[:],
                        lhsT=wih[0][:, ks, m * 128:(m + 1) * 128],
                        rhs=Xgru[:, ks, b0:b1, :],
                        start=(ks == 0), stop=(ks == 1))
                if zero_bias:
                    nc.vector.tensor_copy(GIb[:, m, b0:b1, :], pg[:])
                else:
                    nc.vector.tensor_scalar_add(
                        GIb[:, m, b0:b1, :], pg[:], bih_sb[0][:, m:m + 1])

        # emission order IS per-engine execution order: phase-2 of quarter
        # q sits between phase-1 quarters so its PE/DVE work runs under
        # the DMA of later batches.  The sigmoid/tanh ACT-table warm-up is
        # emitted right after the LAST quarter's relu (= after the last
        # exp on the ACT queue) so the ~2.7us table switch overlaps the
        # final gi matmuls/copies instead of serializing after them.
        for q in range(NQ):
            for b in range(q * QB, (q + 1) * QB):
                phase1(b)
            phase2(p2, pp, q)
            if q == NQ - 1:
                warm = gs.tile([1, 1], f32, tag="warm", name="warm")
                nc.scalar.activation(warm[:], Xgru[0:1, 0, BL - 1, 0:1],
                                     AF.Sigmoid)
            phase2_gi(p2, pp, q)

        # GRU-only weights: DMA'd after the phase emission so the H
        # stream owns the head of the DMA queues (they land ~65us in,
        # long before first use is possible)
        nc.sync.dma_start(wih[1][:], Wihd[1].ap()[:])
        for i in range(L):
            nc.sync.dma_start(whh[i][:], Whhd[i].ap()[:])
        nc.sync.dma_start(wo[:], Wod.ap()[:])
        nc.sync.dma_start(bo_sb[:], bod.ap()[:])
        nc.sync.dma_start(eye[:], eyed.ap()[:])

        # =========== Phase 3: 2-layer GRU over T steps =====================
        DLT = 6  # layer-2 lag; its gi is bulk-computed per DLT-step block
        GI2 = persist.tile([128, G, BL, 2, DLT], dth)  # 2-slot ring

        def bulk_gi2(k):
            """gi for layer 2, steps [k*DLT, (k+1)*DLT), into ring slot."""
            sl = k % 2
            pg = pp.tile([128, G, BL, DLT], f32, tag="bank", name=f"pg2_{k}")
            for m in range(G):
                for ks in range(2):
                    nc.tensor.matmul(
                        pg[:, m, :, :],
                        lhsT=wih[1][:, ks, m * 128:(m + 1) * 128],
                        rhs=Y1[:, ks, :, k * DLT:(k + 1) * DLT],
                        start=(ks == 0), stop=(ks == 1))
            # PSUM -> SBUF ring copy on the ACT engine: DVE is the GRU's
            # busiest engine, ACT has slack
            nc.scalar.activation(GI2[:, :, :, sl, :], pg[:], AF.Copy)

        def gi_of(li, t):
            return (GIb[:, :, :, t] if li == 0
                    else GI2[:, :, :, (t // DLT) % 2, t % DLT])

        # per-layer live state carried between head/mid/tail emissions
        st = [{}, {}]

        def gru_head(li, t):
            """PE + gate sigmoids for step t (t>0).  Three PSUM banks:
            Pr (gi_r inject + 4 r matmuls), Pn (4 gh_n matmuls; gi_n is
            added from SBUF at dn), Pz (gi_z inject + 4 z matmuls).  PE
            order r -> n -> z, so sig_r fires after just 5 matmuls and
            the n-branch (critical path) starts earliest; sig_z is only
            needed at zd, well after the z matmuls retire."""
            yout = Y1 if li == 0 else Y2
            gisrc = gi_of(li, t)
            Pr = pp.tile([128, 2, BL], f32, tag="bank", name=f"Pr{li}_{t}")
            Pn = pp.tile([128, 2, BL], f32, tag="bank", name=f"Pn{li}_{t}")
            Pz = pp.tile([128, 2, BL], f32, tag="bank", name=f"Pz{li}_{t}")
            nc.tensor.matmul(Pr[:], lhsT=eye, rhs=gisrc[:, 0:2, :],
                             start=True, stop=False)
            for m in range(2):
                for ks in range(2):
                    nc.tensor.matmul(
                        Pr[:, m, :],
                        lhsT=whh[li][:, ks, m * 128:(m + 1) * 128],
                        rhs=yout[:, ks, :, t - 1],
                        start=False, stop=(m == 1 and ks == 1))
            for j in range(2):
                for ks in range(2):
                    nc.tensor.matmul(
                        Pn[:, j, :],
                        lhsT=whh[li][:, ks, (4 + j) * 128:(5 + j) * 128],
                        rhs=yout[:, ks, :, t - 1],
                        start=(j == 0 and ks == 0),
                        stop=(j == 1 and ks == 1))
            sig = gs.tile([128, 2, BL], f32, tag=f"sig{li}", name=f"sig{li}")
            nc.scalar.activation(sig[:], Pr[:], AF.Sigmoid)
            nc.tensor.matmul(Pz[:], lhsT=eye, rhs=gisrc[:, 2:4, :],
                             start=True, stop=False)
            for m in range(2):
                for ks in range(2):
                    nc.tensor.matmul(
                        Pz[:, m, :],
                        lhsT=whh[li][:, ks, (2 + m) * 128:(3 + m) * 128],
                        rhs=yout[:, ks, :, t - 1],
                        start=False, stop=(m == 1 and ks == 1))
            sigz = gs.tile([128, 2, BL], f32, tag=f"sigz{li}",
                           name=f"sigz{li}")
            nc.scalar.activation(sigz[:], Pz[:], AF.Sigmoid)
            st[li] = {"t": t, "Pn": Pn, "sig": sig, "sigz": sigz}

        def gru_mid(li):
            """cn/dn + tanh for the step whose head already ran."""
            t = st[li]["t"]
            cn = gs.tile([128, 2, BL], f32, tag=f"cn{li}", name=f"cn{li}")
            nc.vector.scalar_tensor_tensor(
                cn[:], st[li]["Pn"][:], 1.0, st[li]["sig"][:],
                op0=OP.bypass, op1=OP.mult)
            dn = gs.tile([128, 2, BL], f32, tag=f"dn{li}", name=f"dn{li}")
            nc.vector.tensor_tensor(dn[:], cn[:], gi_of(li, t)[:, 4:6, :],
                                    OP.add)
            ntn = gs.tile([128, 2, BL], f32, tag=f"ntn{li}", name=f"ntn{li}")
            nc.scalar.activation(ntn[:], dn[:], AF.Tanh)
            st[li]["ntn"] = ntn

        def gru_tail(li):
            """h = n + z*(hprev - n); h written to Y{li} as fp16."""
            t = st[li]["t"]
            yout = Y1 if li == 0 else Y2
            sigz, ntn = st[li]["sigz"], st[li]["ntn"]
            df = gs.tile([128, 2, BL], f32, tag=f"df{li}", name=f"df{li}")
            nc.vector.tensor_sub(df[:], yout[:, :, :, t - 1], ntn[:])
            zd = gs.tile([128, 2, BL], f32, tag=f"zd{li}", name=f"zd{li}")
            nc.vector.tensor_tensor(zd[:], sigz[:], df[:], OP.mult)
            nc.vector.tensor_add(yout[:, :, :, t], ntn[:], zd[:])

        def gru_step0(li):
            """t == 0: no hprev, no gh; gi read straight from SBUF.
            h0 = n - z*n."""
            yout = Y1 if li == 0 else Y2
            gisrc = gi_of(li, 0)
            sigz = gs.tile([128, 2, BL], f32, tag=f"sigz{li}",
                           name=f"sigz{li}")
            nc.scalar.activation(sigz[:], gisrc[:, 2:4, :], AF.Sigmoid)
            ntn = gs.tile([128, 2, BL], f32, tag=f"ntn{li}", name=f"ntn{li}")
            nc.scalar.activation(ntn[:], gisrc[:, 4:6, :], AF.Tanh)
            zn = gs.tile([128, 2, BL], f32, tag=f"zn{li}", name=f"zn{li}")
            nc.vector.tensor_tensor(zn[:], sigz[:], ntn[:], OP.mult)
            nc.vector.tensor_sub(yout[:, :, :, 0], ntn[:], zn[:])

        if zero_bias:
            # Software-pipelined emission: layer 1's step is split across
            # the round boundary (head at round end, mid+tail at the next
            # round's start) so the in-order ACT queue
            # [L1.tanh, L0.sig, L0.tanh, L1.sig] matches readiness order.
            for tt in range(T + DLT + 1):
                t1 = tt - DLT - 1     # L1 step finishing this round
                if t1 == 0:
                    gru_step0(1)
                elif 0 < t1 < T:
                    gru_mid(1)
                    gru_tail(1)
                if tt < T:
                    if tt == 0:
                        gru_step0(0)
                    else:
                        gru_head(0, tt)
                        gru_mid(0)
                        gru_tail(0)
                th = tt - DLT         # L1 head for next round's mid/tail
                if 0 < th < T:
                    gru_head(1, th)
                # bulk gi2 last: its PE burst then runs while the next
                # round's chains are in ACT/DVE-land
                if tt < T and tt % DLT == DLT - 1:
                    bulk_gi2(tt // DLT)
        else:
          with tc.tile_pool(name="g", bufs=1) as gp, \
             tc.tile_pool(name="ppg", bufs=6, space="PSUM") as ppg:
            for li in range(L):
                xin = Xgru if li == 0 else Y1
                yout = Y1 if li == 0 else Y2
                # bulk gi = W_ih @ x (+ b_ih)
                for m in range(G):
                    pg = ppg.tile([128, BT], f32, tag="gbank", name=f"pg{li}{m}")
                    for ks in range(2):
                        nc.tensor.matmul(
                            pg[:],
                            lhsT=wih[li][:, ks, m * 128:(m + 1) * 128],
                            rhs=xin[:, ks, :, :],
                            start=(ks == 0), stop=(ks == 1))
                    if zero_bias:
                        nc.vector.tensor_copy(GIb[:, m, :, :], pg[:])
                    else:
                        nc.vector.tensor_scalar_add(
                            GIb[:, m, :, :], pg[:], bih_sb[li][:, m:m + 1])
                hprev = None
                for t in range(T):
                    git = GIb[:, :, :, t]
                    if t == 0:
                        if zero_bias:
                            sig = gs.tile([128, 4, BL], f32, tag="sig")
                            nc.scalar.activation(sig[:], git[:, 0:4, :],
                                                 AF.Sigmoid)
                            ntn = gs.tile([128, 2, BL], f32, tag="ntn")
                            nc.scalar.activation(ntn[:], git[:, 4:6, :],
                                                 AF.Tanh)
                        else:
                            arz = gs.tile([128, 4, BL], f32, tag="arz")
                            for m in range(4):
                                nc.vector.tensor_scalar_add(
                                    arz[:, m, :], git[:, m, :],
                                    bhh_sb[li][:, m:m + 1])
                            sig = gs.tile([128, 4, BL], f32, tag="sig")
                            nc.scalar.activation(sig[:], arz[:], AF.Sigmoid)
                            dn = gs.tile([128, 2, BL], f32, tag="dn")
                            for j in range(2):
                                # gi_n + r*b_hh_n
                                nc.vector.scalar_tensor_tensor(
                                    dn[:, j, :], sig[:, j, :],
                                    bhh_sb[li][:, 4 + j:5 + j], git[:, 4 + j, :],
                                    op0=OP.mult, op1=OP.add)
                            ntn = gs.tile([128, 2, BL], f32, tag="ntn")
                            nc.scalar.activation(ntn[:], dn[:], AF.Tanh)
                        # h1 = n - z*n
                        zn = gs.tile([128, 2, BL], f32, tag="zn")
                        nc.vector.tensor_tensor(
                            zn[:], sig[:, 2:4, :], ntn[:], OP.mult)
                        hcur = gs.tile([128, 2, BL], f32, tag="hf32")
                        nc.vector.tensor_sub(hcur[:], ntn[:], zn[:])
                    else:
                        P = ppg.tile([128, G, BL], f32, tag="gbank",
                                     name=f"P{li}_{t}")
                        for m in range(G):
                            for ks in range(2):
                                nc.tensor.matmul(
                                    P[:, m, :],
                                    lhsT=whh[li][:, ks, m * 128:(m + 1) * 128],
                                    rhs=yout[:, ks, :, t - 1],
                                    start=(ks == 0), stop=(ks == 1))
                        arz = gs.tile([128, 4, BL], f32, tag="arz")
                        if zero_bias:
                            nc.vector.tensor_add(
                                arz[:], P[:, 0:4, :], git[:, 0:4, :])
                        else:
                            for m in range(4):
                                nc.vector.scalar_tensor_tensor(
                                    arz[:, m, :], P[:, m, :],
                                    bhh_sb[li][:, m:m + 1], git[:, m, :],
                                    op0=OP.add, op1=OP.add)
                        sig = gs.tile([128, 4, BL], f32, tag="sig")
                        nc.scalar.activation(sig[:], arz[:], AF.Sigmoid)
                        # n = tanh(gi_n + r * (gh_n + b_hh_n))
                        cn = gs.tile([128, 2, BL], f32, tag="cn")
                        if zero_bias:
                            nc.vector.scalar_tensor_tensor(
                                cn[:], P[:, 4:6, :], 1.0, sig[:, 0:2, :],
                                op0=OP.bypass, op1=OP.mult)
                        else:
                            for j in range(2):
                                nc.vector.scalar_tensor_tensor(
                                    cn[:, j, :], P[:, 4 + j, :],
                                    bhh_sb[li][:, 4 + j:5 + j], sig[:, j, :],
                                    op0=OP.add, op1=OP.mult)
                        dn = gs.tile([128, 2, BL], f32, tag="dn")
                        nc.vector.tensor_add(dn[:], cn[:], git[:, 4:6, :])
                        ntn = gs.tile([128, 2, BL], f32, tag="ntn")
                        nc.scalar.activation(ntn[:], dn[:], AF.Tanh)
                        # h = n + z*(hprev - n)
                        df = gs.tile([128, 2, BL], f32, tag="df")
                        nc.vector.tensor_sub(df[:], hprev[:], ntn[:])
                        zd = gs.tile([128, 2, BL], f32, tag="zd")
                        nc.vector.tensor_tensor(
                            zd[:], sig[:, 2:4, :], df[:], OP.mult)
                        hcur = gs.tile([128, 2, BL], f32, tag="hf32")
                        nc.vector.tensor_add(hcur[:], ntn[:], zd[:])
                    nc.vector.tensor_copy(yout[:, :, :, t], hcur[:])
                    hprev = hcur

        # final: relu(y2_last @ Wo.T + bo)
        po = pp.tile([BL, 1], f32, tag="bank", name="po")
        for ks in range(2):
            nc.tensor.matmul(po[:], lhsT=Y2[:, ks, :, T - 1],
                             rhs=wo[:, ks, :], start=(ks == 0),
                             stop=(ks == 1))
        osb = gs.tile([BL, 1], f32, tag="osb", name="osb")
        if os.environ.get("BASS_DEC_RAW"):
            # debug: skip the final relu so the output is informative
            nc.vector.tensor_scalar_add(osb[:], po[:], bo_sb[:, 0:1])
        else:
            nc.scalar.activation(osb[:], po[:], AF.Relu, bias=bo_sb[:, 0:1])
        nc.sync.dma_start(outd.ap()[:], osb[:])

    nc.compile()
    return nc


def _prep_inputs(inputs, prec):
    """Host-side: sharding + device-ready layouts."""
    npdt = _np_dt(prec)
    npds = _np_stream_dt(prec)
    H = np.asarray(inputs["H"], np.float32)
    l = np.asarray(inputs["l"], np.float32)
    knn = np.argsort(l, axis=-1)[:, -3:]                       # [B, 3]
    S = np.zeros((B, N, 4), np.float32)
    S[:, :, 0] = l
    bi = np.arange(B)[:, None]
    for k in range(3):
        S[bi[:, 0], knn[:, k], k + 1] = 1.0

    def wT(w, nslice):  # [fo, fi] -> [128, nslice, fo] with fi=ks*128+p
        w = np.asarray(w, np.float32)
        return np.ascontiguousarray(
            w.T.reshape(nslice, 128, w.shape[0]).transpose(1, 0, 2))

    def bcol(bvec, nslice):  # [P] -> [128, nslice]
        return np.ascontiguousarray(
            np.asarray(bvec, np.float32).reshape(nslice, 128).T)

    wq = wT(inputs["Wq"], 2).astype(npdt)
    wk = wT(inputs["Wk"], 2).astype(npdt)
    wv = wT(inputs["Wv"], 2).astype(npdt)
    wkk = wT(inputs["Wkk"], 4).astype(npdt)
    wih = [wT(np.asarray(inputs["gru_w_ih"])[i], 2).astype(npdt)
           for i in range(L)]
    whh = [wT(np.asarray(inputs["gru_w_hh"])[i], 2).astype(npdt)
           for i in range(L)]
    wo = wT(inputs["Wo"], 2).astype(npdt)
    bq = bcol(inputs["bq"], 2)
    bk = bcol(inputs["bk"], 2)
    bv = bcol(inputs["bv"], 2)
    bkk = bcol(inputs["bkk"], 2)
    bih = [bcol(np.asarray(inputs["gru_b_ih"])[i], G) for i in range(L)]
    bhh = [bcol(np.asarray(inputs["gru_b_hh"])[i], G) for i in range(L)]
    bo = np.full((BL, 1), np.float32(np.asarray(inputs["bo"])[0]))

    zero_bias = all(
        not np.any(np.asarray(inputs[k]))
        for k in ("bq", "bk", "bv", "bkk", "gru_b_ih", "gru_b_hh", "bo"))

    # H -> [B, N, T, F] then per-core shards
    Ht = np.ascontiguousarray(H.transpose(0, 2, 1, 3)).astype(npds)
    in_maps = []
    for c in range(NCORES):
        sl = slice(c * BL, (c + 1) * BL)
        m = {
            "H": np.ascontiguousarray(Ht[sl]),
            "S": np.ascontiguousarray(
                S[sl].transpose(1, 0, 2)).astype(npds),
            "WqT": wq, "WkT": wk, "WvT": wv, "WkkT": wkk,
            "WoT": wo, "bq": bq, "bk": bk, "bv": bv, "bkk": bkk, "bo": bo,
            "EYE": np.eye(128, dtype=npdt),
        }
        for i in range(L):
            m[f"WihT{i}"] = wih[i]
            m[f"WhhT{i}"] = whh[i]
            m[f"bih{i}"] = bih[i]
            m[f"bhh{i}"] = bhh[i]
        in_maps.append(m)
    return in_maps, zero_bias


def _ensure_ntff_hook():
    """The agent image's antenv lacks axon_hooks; synthesize it and
    register the ctypes NTFF hook from trn_agent_boot."""
    import types

    try:
        from antenv import axon_hooks  # noqa: F401
        return
    except ImportError:
        pass
    import antenv

    mod = types.ModuleType("antenv.axon_hooks")
    _h = [None]
    mod.set_axon_ntff_profile_hook = lambda h: _h.__setitem__(0, h)
    mod.get_axon_ntff_profile_hook = lambda: _h[0]
    sys.modules["antenv.axon_hooks"] = mod
    antenv.axon_hooks = mod
    try:
        from trn_agent_boot.trn_boot import _ntff_profile_via_ctypes

        h = _ntff_profile_via_ctypes("/opt/axon/libaxon_pjrt.so")
        if h is not None:
            mod.set_axon_ntff_profile_hook(h)
    except Exception as e:  # pragma: no cover
        print("ntff hook install failed:", e)


def run(inputs, prec=None, trace=False):
    prec = prec or _PREC
    in_maps, zero_bias = _prep_inputs(inputs, prec)
    key = (zero_bias, prec)
    if key not in _NC_CACHE:
        _NC_CACHE[key] = _build(zero_bias, prec)
    nc = _NC_CACHE[key]
    if trace:
        _ensure_ntff_hook()
    from concourse.bass_utils import run_bass_kernel_spmd
    res = run_bass_kernel_spmd(nc, in_maps, list(range(NCORES)), trace=trace)
    out = np.concatenate([res.results[c]["out"] for c in range(NCORES)], 0)
    return np.ascontiguousarray(out, dtype=np.float32), res


def kernel(**inputs) -> np.ndarray:
    out, _ = run(inputs)
    return out



# revision 24
# speedup vs baseline: 1.0105x; 1.0105x over previous
"""Trainium2 Bass kernel for nn_Decoder (sparse_attention).

Reference computation (per batch b):
  knn   = top-3 stations by l[b]                         (sparse attention support)
  q_in  = sum_n l[b,n] * H[b,t,n,:]                      [T,F]
  q     = q_in @ Wq.T + bq
  keys  = H @ Wk.T + bk   (only needed at knn stations)
  attn  = softmax over the 3 knn stations of q . keys
  vals  = H @ Wv.T + bv   (only needed at knn stations)
  h_kn  = sum_k attn_k * vals_k = Wv @ (sum_k attn_k * Hsel_k) + bv
  h     = relu(concat([q_in, h_kn]) @ Wkk.T + bkk)
  x     = GRU_2layer(h); out = relu(x[:,-1,:] @ Wo.T + bo)

Kernel strategy (8 cores, data-parallel over batch, 8 batches/core):
  Phase 1: stream H[b] tiles [n=128, t*F] through the PE as the stationary
    operand against a small selection matrix S_b [128, 4] whose columns are
    (l[b], onehot(knn0), onehot(knn1), onehot(knn2)).  One pass over H
    produces both q_in and the 3 gathered stations with F on partitions.
  Phase 2: per quarter-batch (96 (b,t) columns), pipelined under the
    phase-1 DMA of later quarters: q/keys projections, scores via
    elementwise-mul + ones-matmul partition reduction, 3-way softmax,
    attn broadcast via ones-matmul, station mix, Wv and Wkk projections,
    relu, and the layer-1 bulk gi.  All projection matmuls run in fp16
    (1 cycle/row vs fp32's 4).
  Phase 3: 2-layer GRU, software-pipelined: layer 1's step is split
    across the round boundary (head at round end, mid+tail at next round
    start) so the in-order ACT queue matches readiness order.  Per step,
    three PSUM banks Pr/Pn/Pz (r-gate inject+matmuls first) let sig_r
    fire after 5 matmuls; gi_n is added from SBUF, sig_z retires off the
    critical path.  Steady round (both layers) ~2.37us on HW.

Precision: H and S stream as fp8 e4m3 (~12.6 MiB/core, halves the fp16
HBM roofline); everything else fp16 with fp32 PSUM accumulation.  The
final ReLU output margin is ~0.013 vs ~2.4e-3 total fp8-induced error
(verified via BASS_DEC_RAW=1).  Env BASS_DEC_PREC=f8|f16|f32 (default f8).
"""

import os
import sys
from contextlib import ExitStack

import numpy as np

for _p in ("/opt/trn_rl_repo", "/root/.axon_site/_ro/trn_rl_repo"):
    if os.path.isdir(_p) and _p not in sys.path:
        sys.path.insert(0, _p)

B, T, N, F, L = 64, 48, 128, 256, 2
NCORES = 8
BL = B // NCORES          # local batch per core
BT = BL * T               # phase-2 column count
TC = 24                   # t-chunk for phase-1 DMA/matmul
G = 6                     # gate row-slices (3F/128)

_PREC = os.environ.get("BASS_DEC_PREC", "f8")
_NC_CACHE = {}


def _np_dt(prec):
    return np.float32 if prec == "f32" else np.float16


def _np_stream_dt(prec):
    if prec == "f8":
        import ml_dtypes
        return ml_dtypes.float8_e4m3fn
    return _np_dt(prec)


def _build(zero_bias, prec):
    from concourse import bacc, tile, mybir

    dt = mybir.dt
    f32 = dt.float32
    dth = dt.float32 if prec == "f32" else dt.float16
    # stream dtype for the big H tensor (and the selection matrix S it is
    # multiplied with): fp8 e4m3 halves the HBM roofline again vs fp16
    dts = dt.float8e4 if prec == "f8" else dth

    AF = mybir.ActivationFunctionType
    OP = mybir.AluOpType

    nc = bacc.Bacc("TRN2", target_bir_lowering=False, debug=False,
                   num_devices=NCORES)

    # ---- DRAM I/O (per-core shard) ----
    Hd = nc.dram_tensor("H", [BL, N, T, F], dts, kind="ExternalInput")
    Sd = nc.dram_tensor("S", [N, BL, 4], dts, kind="ExternalInput")
    Wqd = nc.dram_tensor("WqT", [128, 2, F], dth, kind="ExternalInput")
    Wkd = nc.dram_tensor("WkT", [128, 2, F], dth, kind="ExternalInput")
    Wvd = nc.dram_tensor("WvT", [128, 2, F], dth, kind="ExternalInput")
    Wkkd = nc.dram_tensor("WkkT", [128, 4, F], dth, kind="ExternalInput")
    Wihd = [nc.dram_tensor(f"WihT{i}", [128, 2, 3 * F], dth,
                           kind="ExternalInput") for i in range(L)]
    Whhd = [nc.dram_tensor(f"WhhT{i}", [128, 2, 3 * F], dth,
                           kind="ExternalInput") for i in range(L)]
    Wod = nc.dram_tensor("WoT", [128, 2, 1], dth, kind="ExternalInput")
    bqd = nc.dram_tensor("bq", [128, 2], f32, kind="ExternalInput")
    bkd = nc.dram_tensor("bk", [128, 2], f32, kind="ExternalInput")
    bvd = nc.dram_tensor("bv", [128, 2], f32, kind="ExternalInput")
    bkkd = nc.dram_tensor("bkk", [128, 2], f32, kind="ExternalInput")
    bihd = [nc.dram_tensor(f"bih{i}", [128, G], f32, kind="ExternalInput")
            for i in range(L)]
    bhhd = [nc.dram_tensor(f"bhh{i}", [128, G], f32, kind="ExternalInput")
            for i in range(L)]
    bod = nc.dram_tensor("bo", [BL, 1], f32, kind="ExternalInput")
    eyed = nc.dram_tensor("EYE", [128, 128], dth, kind="ExternalInput")
    outd = nc.dram_tensor("out", [BL, 1], f32, kind="ExternalOutput")

    with tile.TileContext(nc) as tc, ExitStack() as ctx:
        cpool = ctx.enter_context(tc.tile_pool(name="consts", bufs=1))
        persist = ctx.enter_context(tc.tile_pool(name="persist", bufs=1))

        # ---- load parameters to SBUF ----
        sS = cpool.tile([N, BL, 4], dts)
        nc.sync.dma_start(sS[:], Sd.ap()[:])
        wq = cpool.tile([128, 2, F], dth)
        nc.sync.dma_start(wq[:], Wqd.ap()[:])
        wk = cpool.tile([128, 2, F], dth)
        nc.sync.dma_start(wk[:], Wkd.ap()[:])
        wv = cpool.tile([128, 2, F], dth)
        nc.sync.dma_start(wv[:], Wvd.ap()[:])
        wkk = cpool.tile([128, 4, F], dth)
        nc.sync.dma_start(wkk[:], Wkkd.ap()[:])
        wih = []
        whh = []
        for i in range(L):
            wih_i = cpool.tile([128, 2, 3 * F], dth, name=f"wih{i}")
            wih.append(wih_i)
            whh_i = cpool.tile([128, 2, 3 * F], dth, name=f"whh{i}")
            whh.append(whh_i)
        # wih[0] feeds the phase-2 gi bulk: load it up front.  The rest of
        # the GRU-only weights are DMA'd after the phase emission (below)
        # so the H stream starts at the head of the DMA queues.
        nc.sync.dma_start(wih[0][:], Wihd[0].ap()[:])
        wo = cpool.tile([128, 2, 1], dth)
        bo_sb = cpool.tile([BL, 1], f32)
        if not zero_bias:
            bq_sb = cpool.tile([128, 2], f32)
            nc.sync.dma_start(bq_sb[:], bqd.ap()[:])
            bk_sb = cpool.tile([128, 2], f32)
            nc.sync.dma_start(bk_sb[:], bkd.ap()[:])
            bv_sb = cpool.tile([128, 2], f32)
            nc.sync.dma_start(bv_sb[:], bvd.ap()[:])
            bkk_sb = cpool.tile([128, 2], f32)
            nc.sync.dma_start(bkk_sb[:], bkkd.ap()[:])
            bih_sb = []
            bhh_sb = []
            for i in range(L):
                bih_i = cpool.tile([128, G], f32, name=f"bih_sb{i}")
                nc.sync.dma_start(bih_i[:], bihd[i].ap()[:])
                bih_sb.append(bih_i)
                bhh_i = cpool.tile([128, G], f32, name=f"bhh_sb{i}")
                nc.sync.dma_start(bhh_i[:], bhhd[i].ap()[:])
                bhh_sb.append(bhh_i)

        ones_col = cpool.tile([128, 1], f32)      # scores reduction lhsT
        nc.gpsimd.memset(ones_col[:], 1.0)
        ones_row = cpool.tile([1, 128], f32)      # broadcast lhsT
        nc.gpsimd.memset(ones_row[:], 1.0)
        eye = cpool.tile([128, 128], dth)         # identity: psum-inject lhsT

        # X[p, s, b, t, c]: c=0 -> q_in, c=1..3 -> selected stations
        # X split per QUARTER-batch so phase 2 of quarter q starts while
        # phase-1 DMA of quarter q+1 is still streaming (Tile deps are
        # whole-tile, not per-slice).  fp16: phase-2 matmuls then run at
        # 1 cycle/row instead of fp32's 4.
        NQ = 4
        QB = BL // NQ
        Xh = [persist.tile([128, 2, QB, T, 4], dth, name=f"X{q}")
              for q in range(NQ)]
        Xgru = persist.tile([128, 2, BL, T], dth)   # phase-2 output h
        # bulk gi for layer 1 (fp16 in the fast path: re-injected into
        # PSUM by an identity matmul each step)
        GIb = persist.tile([128, G, BL, T], dth if zero_bias else f32)
        Y1 = persist.tile([128, 2, BL, T], dth)
        Y2 = persist.tile([128, 2, BL, T], dth)

        # one shared PSUM pool for all phases: 8 rotating bank slots, so
        # phases pipeline instead of serializing on pool address reuse
        pp = ctx.enter_context(tc.tile_pool(name="pp", bufs=8, space="PSUM"))
        hp = ctx.enter_context(tc.tile_pool(name="hload", bufs=8))
        p2 = ctx.enter_context(tc.tile_pool(name="p2", bufs=1))
        gs = ctx.enter_context(tc.tile_pool(name="gs", bufs=3))

        # =========== Phase 1: q_in + knn gather (one pass over H) ==========
        def phase1(b):
            for tci in range(T // TC):
                ht = hp.tile([128, TC, F], dts, tag="ht", name="ht")
                nc.sync.dma_start(
                    ht[:], Hd.ap()[b, :, tci * TC:(tci + 1) * TC, :])
                pt = pp.tile([128, 2, TC, 4], f32, tag="bank", name="pt")
                for s in range(2):
                    for ti in range(TC):
                        nc.tensor.matmul(
                            pt[:, s, ti, :],
                            lhsT=ht[:, ti, s * 128:(s + 1) * 128],
                            rhs=sS[:, b, :],
                            start=True, stop=True)
                nc.vector.tensor_copy(
                    Xh[b // QB][:, :, b % QB, tci * TC:(tci + 1) * TC, :],
                    pt[:])

        # =========== Phase 2: attention + mix + mlp ========================
        # done per quarter-batch so it overlaps phase-1 DMA of later batches
        def phase2(p2, pp2, q):
            b0, b1 = q * QB, (q + 1) * QB
            nb = QB * T
            XH = Xh[q]
            rhs_qin = XH[:, :, :, :, 0]
            prodS = p2.tile([128, 3, 2, nb], f32, tag="prodS", bufs=2,
                            name=f"prodS{q}")
            pq = []
            for ms in range(2):
                pq_ms = pp2.tile([128, nb], f32, tag="bank",
                                 name=f"pq{q}{ms}")
                for ks in range(2):
                    nc.tensor.matmul(
                        pq_ms[:],
                        lhsT=wq[:, ks, ms * 128:(ms + 1) * 128],
                        rhs=rhs_qin[:, ks],
                        start=(ks == 0), stop=(ks == 1))
                pq.append(pq_ms)
            for k in range(3):
                for ms in range(2):
                    pk = pp2.tile([128, nb], f32, tag="bank",
                                  name=f"pk{q}{k}{ms}")
                    for ks in range(2):
                        nc.tensor.matmul(
                            pk[:],
                            lhsT=wk[:, ks, ms * 128:(ms + 1) * 128],
                            rhs=XH[:, ks, :, :, k + 1],
                            start=(ks == 0), stop=(ks == 1))
                    ksb = p2.tile([128, nb], f32, tag="ksb", bufs=2,
                                  name=f"ksb{q}{k}{ms}")
                    if zero_bias:
                        nc.vector.tensor_copy(ksb[:], pk[:])
                        nc.vector.tensor_tensor(
                            prodS[:, k, ms, :], ksb[:], pq[ms][:], OP.mult)
                    else:
                        nc.vector.tensor_scalar_add(
                            ksb[:], pk[:], bk_sb[:, ms:ms + 1])
                        nc.vector.scalar_tensor_tensor(
                            prodS[:, k, ms, :], pq[ms][:],
                            bq_sb[:, ms:ms + 1], ksb[:],
                            op0=OP.add, op1=OP.mult)
            psc = []
            for k in range(3):
                ps = pp2.tile([1, nb], f32, tag="bank", name=f"ps{q}{k}")
                for ms in range(2):
                    nc.tensor.matmul(
                        ps[:], lhsT=ones_col[:, 0:1], rhs=prodS[:, k, ms, :],
                        start=(ms == 0), stop=(ms == 1))
                psc.append(ps)
            E = p2.tile([1, 3, nb], f32, tag="E", bufs=2, name=f"E{q}")
            for k in range(3):
                nc.scalar.activation(E[:, k, :], psc[k][:], AF.Exp)
            s2 = p2.tile([1, nb], f32, tag="s2", bufs=2, name=f"s2_{q}")
            nc.vector.tensor_add(s2[:], E[:, 0, :], E[:, 1, :])
            ssum = p2.tile([1, nb], f32, tag="ssum", bufs=2, name=f"ssum{q}")
            nc.vector.tensor_add(ssum[:], s2[:], E[:, 2, :])
            rec = p2.tile([1, nb], f32, tag="rec", bufs=2, name=f"rec{q}")
            nc.vector.reciprocal(rec[:], ssum[:])
            attn = p2.tile([1, 3, nb], f32, tag="attn", bufs=2,
                           name=f"attn{q}")
            for k in range(3):
                nc.vector.tensor_tensor(
                    attn[:, k, :], E[:, k, :], rec[:], OP.mult)
            pb = []
            for k in range(3):
                pb_k = pp2.tile([128, nb], f32, tag="bank",
                                name=f"pb{q}{k}")
                nc.tensor.matmul(pb_k[:], lhsT=ones_row[0:1, :],
                                 rhs=attn[:, k, :], start=True, stop=True)
                pb.append(pb_k)
            hm = p2.tile([128, 2, nb], dth, tag="hm", bufs=2, name=f"hm{q}")
            for s in range(2):
                m0 = p2.tile([128, nb], f32, tag="mixt", bufs=2,
                             name=f"m0_{q}{s}")
                nc.vector.tensor_tensor(
                    m0[:], pb[0][:], XH[:, s, :, :, 1], OP.mult)
                m1 = p2.tile([128, nb], f32, tag="mixt", bufs=2,
                             name=f"m1_{q}{s}")
                nc.vector.tensor_tensor(
                    m1[:], pb[1][:], XH[:, s, :, :, 2], OP.mult)
                a0 = p2.tile([128, nb], f32, tag="mixa", bufs=2,
                             name=f"a0_{q}{s}")
                nc.vector.tensor_add(a0[:], m0[:], m1[:])
                m2 = p2.tile([128, nb], f32, tag="mixt", bufs=2,
                             name=f"m2_{q}{s}")
                nc.vector.tensor_tensor(
                    m2[:], pb[2][:], XH[:, s, :, :, 3], OP.mult)
                nc.vector.tensor_add(hm[:, s, :], a0[:], m2[:])
            vsb = p2.tile([128, 2, nb], dth, tag="vsb", bufs=2,
                          name=f"vsb{q}")
            for ms in range(2):
                pv = pp2.tile([128, nb], f32, tag="bank",
                              name=f"pv{q}{ms}")
                for ks in range(2):
                    nc.tensor.matmul(
                        pv[:], lhsT=wv[:, ks, ms * 128:(ms + 1) * 128],
                        rhs=hm[:, ks, :], start=(ks == 0), stop=(ks == 1))
                if zero_bias:
                    nc.vector.tensor_copy(vsb[:, ms, :], pv[:])
                else:
                    nc.vector.tensor_scalar_add(
                        vsb[:, ms, :], pv[:], bv_sb[:, ms:ms + 1])
            for ms in range(2):
                ph = pp2.tile([128, nb], f32, tag="bank",
                              name=f"ph{q}{ms}")
                for ks in range(4):
                    rhs = rhs_qin[:, ks] if ks < 2 else vsb[:, ks - 2, :]
                    nc.tensor.matmul(
                        ph[:], lhsT=wkk[:, ks, ms * 128:(ms + 1) * 128],
                        rhs=rhs, start=(ks == 0), stop=(ks == 3))
                bias = 0.0 if zero_bias else bkk_sb[:, ms:ms + 1]
                nc.scalar.activation(Xgru[:, ms, b0:b1, :], ph[:], AF.Relu,
                                     bias=bias)

        # layer-1 bulk gi for this quarter while DMA continues
        def phase2_gi(p2, pp2, q):
            b0, b1 = q * QB, (q + 1) * QB
            nb = QB * T
            for m in range(G):
                pg = pp2.tile([128, nb], f32, tag="bank",
                              name=f"pg{q}{m}")
                for ks in range(2):
                    nc.tensor.matmul(
                        pg[:],
                        lhsT=wih[0][:, ks, m * 128:(m + 1) * 128],
                        rhs=Xgru[:, ks, b0:b1, :],
                        start=(ks == 0), stop=(ks == 1))
                if zero_bias:
                    nc.vector.tensor_copy(GIb[:, m, b0:b1, :], pg[:])
                else:
                    nc.vector.tensor_scalar_add(
                        GIb[:, m, b0:b1, :], pg[:], bih_sb[0][:, m:m + 1])

        # emission order IS per-engine execution order: phase-2 of quarter
        # q sits between phase-1 quarters so its PE/DVE work runs under
        # the DMA of later batches.  The sigmoid/tanh ACT-table warm-up is
        # emitted right after the LAST quarter's relu (= after the last
        # exp on the ACT queue) so the ~2.7us table switch overlaps the
        # final gi matmuls/copies instead of serializing after them.
        for q in range(NQ):
            for b in range(q * QB, (q + 1) * QB):
                phase1(b)
            phase2(p2, pp, q)
            if q == NQ - 1:
                warm = gs.tile([1, 1], f32, tag="warm", name="warm")
                nc.scalar.activation(warm[:], Xgru[0:1, 0, BL - 1, 0:1],
                                     AF.Sigmoid)
            phase2_gi(p2, pp, q)

        # GRU-only weights: DMA'd after the phase emission so the H
        # stream owns the head of the DMA queues (they land ~65us in,
        # long before first use is possible)
        nc.sync.dma_start(wih[1][:], Wihd[1].ap()[:])
        for i in range(L):
            nc.sync.dma_start(whh[i][:], Whhd[i].ap()[:])
        nc.sync.dma_start(wo[:], Wod.ap()[:])
        nc.sync.dma_start(bo_sb[:], bod.ap()[:])
        nc.sync.dma_start(eye[:], eyed.ap()[:])

        # =========== Phase 3: 2-layer GRU over T steps =====================
        DLT = 3  # layer-2 lag; its gi is bulk-computed per DLT-step block
        GI2 = persist.tile([128, G, BL, 2, DLT], dth)  # 2-slot ring

        def bulk_gi2(k):
            """gi for layer 2, steps [k*DLT, (k+1)*DLT), into ring slot."""
            sl = k % 2
            pg = pp.tile([128, G, BL, DLT], f32, tag="bank", name=f"pg2_{k}")
            for m in range(G):
                for ks in range(2):
                    nc.tensor.matmul(
                        pg[:, m, :, :],
                        lhsT=wih[1][:, ks, m * 128:(m + 1) * 128],
                        rhs=Y1[:, ks, :, k * DLT:(k + 1) * DLT],
                        start=(ks == 0), stop=(ks == 1))
            # PSUM -> SBUF ring copy on the ACT engine: DVE is the GRU's
            # busiest engine, ACT has slack
            nc.scalar.activation(GI2[:, :, :, sl, :], pg[:], AF.Copy)

        def gi_of(li, t):
            return (GIb[:, :, :, t] if li == 0
                    else GI2[:, :, :, (t // DLT) % 2, t % DLT])

        # per-layer live state carried between head/mid/tail emissions
        st = [{}, {}]

        def gru_head(li, t):
            """PE + gate sigmoids for step t (t>0).  Three PSUM banks:
            Pr (gi_r inject + 4 r matmuls), Pn (4 gh_n matmuls; gi_n is
            added from SBUF at dn), Pz (gi_z inject + 4 z matmuls).  PE
            order r -> n -> z, so sig_r fires after just 5 matmuls and
            the n-branch (critical path) starts earliest; sig_z is only
            needed at zd, well after the z matmuls retire."""
            yout = Y1 if li == 0 else Y2
            gisrc = gi_of(li, t)
            Pr = pp.tile([128, 2, BL], f32, tag="bank", name=f"Pr{li}_{t}")
            Pn = pp.tile([128, 2, BL], f32, tag="bank", name=f"Pn{li}_{t}")
            Pz = pp.tile([128, 2, BL], f32, tag="bank", name=f"Pz{li}_{t}")
            nc.tensor.matmul(Pr[:], lhsT=eye, rhs=gisrc[:, 0:2, :],
                             start=True, stop=False)
            for m in range(2):
                for ks in range(2):
                    nc.tensor.matmul(
                        Pr[:, m, :],
                        lhsT=whh[li][:, ks, m * 128:(m + 1) * 128],
                        rhs=yout[:, ks, :, t - 1],
                        start=False, stop=(m == 1 and ks == 1))
            for j in range(2):
                for ks in range(2):
                    nc.tensor.matmul(
                        Pn[:, j, :],
                        lhsT=whh[li][:, ks, (4 + j) * 128:(5 + j) * 128],
                        rhs=yout[:, ks, :, t - 1],
                        start=(j == 0 and ks == 0),
                        stop=(j == 1 and ks == 1))
            sig = gs.tile([128, 2, BL], f32, tag=f"sig{li}", name=f"sig{li}")
            nc.scalar.activation(sig[:], Pr[:], AF.Sigmoid)
            nc.tensor.matmul(Pz[:], lhsT=eye, rhs=gisrc[:, 2:4, :],
                             start=True, stop=False)
            for m in range(2):
                for ks in range(2):
                    nc.tensor.matmul(
                        Pz[:, m, :],
                        lhsT=whh[li][:, ks, (2 + m) * 128:(3 + m) * 128],
                        rhs=yout[:, ks, :, t - 1],
                        start=False, stop=(m == 1 and ks == 1))
            sigz = gs.tile([128, 2, BL], f32, tag=f"sigz{li}",
                           name=f"sigz{li}")
            nc.scalar.activation(sigz[:], Pz[:], AF.Sigmoid)
            st[li] = {"t": t, "Pn": Pn, "sig": sig, "sigz": sigz}

        def gru_mid(li):
            """cn/dn + tanh for the step whose head already ran."""
            t = st[li]["t"]
            cn = gs.tile([128, 2, BL], f32, tag=f"cn{li}", name=f"cn{li}")
            nc.vector.scalar_tensor_tensor(
                cn[:], st[li]["Pn"][:], 1.0, st[li]["sig"][:],
                op0=OP.bypass, op1=OP.mult)
            dn = gs.tile([128, 2, BL], f32, tag=f"dn{li}", name=f"dn{li}")
            nc.vector.tensor_tensor(dn[:], cn[:], gi_of(li, t)[:, 4:6, :],
                                    OP.add)
            ntn = gs.tile([128, 2, BL], f32, tag=f"ntn{li}", name=f"ntn{li}")
            nc.scalar.activation(ntn[:], dn[:], AF.Tanh)
            st[li]["ntn"] = ntn

        def gru_tail(li):
            """h = n + z*(hprev - n); h written to Y{li} as fp16."""
            t = st[li]["t"]
            yout = Y1 if li == 0 else Y2
            sigz, ntn = st[li]["sigz"], st[li]["ntn"]
            df = gs.tile([128, 2, BL], f32, tag=f"df{li}", name=f"df{li}")
            nc.vector.tensor_sub(df[:], yout[:, :, :, t - 1], ntn[:])
            zd = gs.tile([128, 2, BL], f32, tag=f"zd{li}", name=f"zd{li}")
            nc.vector.tensor_tensor(zd[:], sigz[:], df[:], OP.mult)
            nc.vector.tensor_add(yout[:, :, :, t], ntn[:], zd[:])

        def gru_step0(li):
            """t == 0: no hprev, no gh; gi read straight from SBUF.
            h0 = n - z*n."""
            yout = Y1 if li == 0 else Y2
            gisrc = gi_of(li, 0)
            sigz = gs.tile([128, 2, BL], f32, tag=f"sigz{li}",
                           name=f"sigz{li}")
            nc.scalar.activation(sigz[:], gisrc[:, 2:4, :], AF.Sigmoid)
            ntn = gs.tile([128, 2, BL], f32, tag=f"ntn{li}", name=f"ntn{li}")
            nc.scalar.activation(ntn[:], gisrc[:, 4:6, :], AF.Tanh)
            zn = gs.tile([128, 2, BL], f32, tag=f"zn{li}", name=f"zn{li}")
            nc.vector.tensor_tensor(zn[:], sigz[:], ntn[:], OP.mult)
            nc.vector.tensor_sub(yout[:, :, :, 0], ntn[:], zn[:])

        if zero_bias:
            # Software-pipelined emission: layer 1's step is split across
            # the round boundary (head at round end, mid+tail at the next
            # round's start) so the in-order ACT queue
            # [L1.tanh, L0.sig, L0.tanh, L1.sig] matches readiness order.
            for tt in range(T + DLT + 1):
                t1 = tt - DLT - 1     # L1 step finishing this round
                if t1 == 0:
                    gru_step0(1)
                elif 0 < t1 < T:
                    gru_mid(1)
                    gru_tail(1)
                if tt < T:
                    if tt == 0:
                        gru_step0(0)
                    else:
                        gru_head(0, tt)
                        gru_mid(0)
                        gru_tail(0)
                th = tt - DLT         # L1 head for next round's mid/tail
                if 0 < th < T:
                    gru_head(1, th)
                # bulk gi2 last: its PE burst then runs while the next
                # round's chains are in ACT/DVE-land
                if tt < T and tt % DLT == DLT - 1:
                    bulk_gi2(tt // DLT)
        else:
          with tc.tile_pool(name="g", bufs=1) as gp, \
             tc.tile_pool(name="ppg", bufs=6, space="PSUM") as ppg:
            for li in range(L):
                xin = Xgru if li == 0 else Y1
                yout = Y1 if li == 0 else Y2
                # bulk gi = W_ih @ x (+ b_ih)
                for m in range(G):
                    pg = ppg.tile([128, BT], f32, tag="gbank", name=f"pg{li}{m}")
                    for ks in range(2):
                        nc.tensor.matmul(
                            pg[:],
                            lhsT=wih[li][:, ks, m * 128:(m + 1) * 128],
                            rhs=xin[:, ks, :, :],
                            start=(ks == 0), stop=(ks == 1))
                    if zero_bias:
                        nc.vector.tensor_copy(GIb[:, m, :, :], pg[:])
                    else:
                        nc.vector.tensor_scalar_add(
                            GIb[:, m, :, :], pg[:], bih_sb[li][:, m:m + 1])
                hprev = None
                for t in range(T):
                    git = GIb[:, :, :, t]
                    if t == 0:
                        if zero_bias:
                            sig = gs.tile([128, 4, BL], f32, tag="sig")
                            nc.scalar.activation(sig[:], git[:, 0:4, :],
                                                 AF.Sigmoid)
                            ntn = gs.tile([128, 2, BL], f32, tag="ntn")
                            nc.scalar.activation(ntn[:], git[:, 4:6, :],
                                                 AF.Tanh)
                        else:
                            arz = gs.tile([128, 4, BL], f32, tag="arz")
                            for m in range(4):
                                nc.vector.tensor_scalar_add(
                                    arz[:, m, :], git[:, m, :],
                                    bhh_sb[li][:, m:m + 1])
                            sig = gs.tile([128, 4, BL], f32, tag="sig")
                            nc.scalar.activation(sig[:], arz[:], AF.Sigmoid)
                            dn = gs.tile([128, 2, BL], f32, tag="dn")
                            for j in range(2):
                                # gi_n + r*b_hh_n
                                nc.vector.scalar_tensor_tensor(
                                    dn[:, j, :], sig[:, j, :],
                                    bhh_sb[li][:, 4 + j:5 + j], git[:, 4 + j, :],
                                    op0=OP.mult, op1=OP.add)
                            ntn = gs.tile([128, 2, BL], f32, tag="ntn")
                            nc.scalar.activation(ntn[:], dn[:], AF.Tanh)
                        # h1 = n - z*n
                        zn = gs.tile([128, 2, BL], f32, tag="zn")
                        nc.vector.tensor_tensor(
                            zn[:], sig[:, 2:4, :], ntn[:], OP.mult)
                        hcur = gs.tile([128, 2, BL], f32, tag="hf32")
                        nc.vector.tensor_sub(hcur[:], ntn[:], zn[:])
                    else:
                        P = ppg.tile([128, G, BL], f32, tag="gbank",
                                     name=f"P{li}_{t}")
                        for m in range(G):
                            for ks in range(2):
                                nc.tensor.matmul(
                                    P[:, m, :],
                                    lhsT=whh[li][:, ks, m * 128:(m + 1) * 128],
                                    rhs=yout[:, ks, :, t - 1],
                                    start=(ks == 0), stop=(ks == 1))
                        arz = gs.tile([128, 4, BL], f32, tag="arz")
                        if zero_bias:
                            nc.vector.tensor_add(
                                arz[:], P[:, 0:4, :], git[:, 0:4, :])
                        else:
                            for m in range(4):
                                nc.vector.scalar_tensor_tensor(
                                    arz[:, m, :], P[:, m, :],
                                    bhh_sb[li][:, m:m + 1], git[:, m, :],
                                    op0=OP.add, op1=OP.add)
                        sig = gs.tile([128, 4, BL], f32, tag="sig")
                        nc.scalar.activation(sig[:], arz[:], AF.Sigmoid)
                        # n = tanh(gi_n + r * (gh_n + b_hh_n))
                        cn = gs.tile([128, 2, BL], f32, tag="cn")
                        if zero_bias:
                            nc.vector.scalar_tensor_tensor(
                                cn[:], P[:, 4:6, :], 1.0, sig[:, 0:2, :],
                                op0=OP.bypass, op1=OP.mult)
                        else:
                            for j in range(2):
                                nc.vector.scalar_tensor_tensor(
                                    cn[:, j, :], P[:, 4 + j, :],
                                    bhh_sb[li][:, 4 + j:5 + j], sig[:, j, :],
                                    op0=OP.add, op1=OP.mult)
                        dn = gs.tile([128, 2, BL], f32, tag="dn")
                        nc.vector.tensor_add(dn[:], cn[:], git[:, 4:6, :])
                        ntn = gs.tile([128, 2, BL], f32, tag="ntn")
                        nc.scalar.activation(ntn[:], dn[:], AF.Tanh)
                        # h = n + z*(hprev - n)
                        df = gs.tile([128, 2, BL], f32, tag="df")
                        nc.vector.tensor_sub(df[:], hprev[:], ntn[:])
                        zd = gs.tile([128, 2, BL], f32, tag="zd")
                        nc.vector.tensor_tensor(
                            zd[:], sig[:, 2:4, :], df[:], OP.mult)
                        hcur = gs.tile([128, 2, BL], f32, tag="hf32")
                        nc.vector.tensor_add(hcur[:], ntn[:], zd[:])
                    nc.vector.tensor_copy(yout[:, :, :, t], hcur[:])
                    hprev = hcur

        # final: relu(y2_last @ Wo.T + bo)
        po = pp.tile([BL, 1], f32, tag="bank", name="po")
        for ks in range(2):
            nc.tensor.matmul(po[:], lhsT=Y2[:, ks, :, T - 1],
                             rhs=wo[:, ks, :], start=(ks == 0),
                             stop=(ks == 1))
        osb = gs.tile([BL, 1], f32, tag="osb", name="osb")
        if os.environ.get("BASS_DEC_RAW"):
            # debug: skip the final relu so the output is informative
            nc.vector.tensor_scalar_add(osb[:], po[:], bo_sb[:, 0:1])
        else:
            nc.scalar.activation(osb[:], po[:], AF.Relu, bias=bo_sb[:, 0:1])
        nc.sync.dma_start(outd.ap()[:], osb[:])

    nc.compile()
    return nc


def _prep_inputs(inputs, prec):
    """Host-side: sharding + device-ready layouts."""
    npdt = _np_dt(prec)
    npds = _np_stream_dt(prec)
    H = np.asarray(inputs["H"], np.float32)
    l = np.asarray(inputs["l"], np.float32)
    knn = np.argsort(l, axis=-1)[:, -3:]                       # [B, 3]
    S = np.zeros((B, N, 4), np.float32)
    S[:, :, 0] = l
    bi = np.arange(B)[:, None]
    for k in range(3):
        S[bi[:, 0], knn[:, k], k + 1] = 1.0

    def wT(w, nslice):  # [fo, fi] -> [128, nslice, fo] with fi=ks*128+p
        w = np.asarray(w, np.float32)
        return np.ascontiguousarray(
            w.T.reshape(nslice, 128, w.shape[0]).transpose(1, 0, 2))

    def bcol(bvec, nslice):  # [P] -> [128, nslice]
        return np.ascontiguousarray(
            np.asarray(bvec, np.float32).reshape(nslice, 128).T)

    wq = wT(inputs["Wq"], 2).astype(npdt)
    wk = wT(inputs["Wk"], 2).astype(npdt)
    wv = wT(inputs["Wv"], 2).astype(npdt)
    wkk = wT(inputs["Wkk"], 4).astype(npdt)
    wih = [wT(np.asarray(inputs["gru_w_ih"])[i], 2).astype(npdt)
           for i in range(L)]
    whh = [wT(np.asarray(inputs["gru_w_hh"])[i], 2).astype(npdt)
           for i in range(L)]
    wo = wT(inputs["Wo"], 2).astype(npdt)
    bq = bcol(inputs["bq"], 2)
    bk = bcol(inputs["bk"], 2)
    bv = bcol(inputs["bv"], 2)
    bkk = bcol(inputs["bkk"], 2)
    bih = [bcol(np.asarray(inputs["gru_b_ih"])[i], G) for i in range(L)]
    bhh = [bcol(np.asarray(inputs["gru_b_hh"])[i], G) for i in range(L)]
    bo = np.full((BL, 1), np.float32(np.asarray(inputs["bo"])[0]))

    zero_bias = all(
        not np.any(np.asarray(inputs[k]))
        for k in ("bq", "bk", "bv", "bkk", "gru_b_ih", "gru_b_hh", "bo"))

    # H -> [B, N, T, F] then per-core shards
    Ht = np.ascontiguousarray(H.transpose(0, 2, 1, 3)).astype(npds)
    in_maps = []
    for c in range(NCORES):
        sl = slice(c * BL, (c + 1) * BL)
        m = {
            "H": np.ascontiguousarray(Ht[sl]),
            "S": np.ascontiguousarray(
                S[sl].transpose(1, 0, 2)).astype(npds),
            "WqT": wq, "WkT": wk, "WvT": wv, "WkkT": wkk,
            "WoT": wo, "bq": bq, "bk": bk, "bv": bv, "bkk": bkk, "bo": bo,
            "EYE": np.eye(128, dtype=npdt),
        }
        for i in range(L):
            m[f"WihT{i}"] = wih[i]
            m[f"WhhT{i}"] = whh[i]
            m[f"bih{i}"] = bih[i]
            m[f"bhh{i}"] = bhh[i]
        in_maps.append(m)
    return in_maps, zero_bias


def _ensure_ntff_hook():
    """The agent image's antenv lacks axon_hooks; synthesize it and
    register the ctypes NTFF hook from trn_agent_boot."""
    import types

    try:
        from antenv import axon_hooks  # noqa: F401
        return
    except ImportError:
        pass
    import antenv

    mod = types.ModuleType("antenv.axon_hooks")
    _h = [None]
    mod.set_axon_ntff_profile_hook = lambda h: _h.__setitem__(0, h)
    mod.get_axon_ntff_profile_hook = lambda: _h[0]
    sys.modules["antenv.axon_hooks"] = mod
    antenv.axon_hooks = mod
    try:
        from trn_agent_boot.trn_boot import _ntff_profile_via_ctypes

        h = _ntff_profile_via_ctypes("/opt/axon/libaxon_pjrt.so")
        if h is not None:
            mod.set_axon_ntff_profile_hook(h)
    except Exception as e:  # pragma: no cover
        print("ntff hook install failed:", e)


def run(inputs, prec=None, trace=False):
    prec = prec or _PREC
    in_maps, zero_bias = _prep_inputs(inputs, prec)
    key = (zero_bias, prec)
    if key not in _NC_CACHE:
        _NC_CACHE[key] = _build(zero_bias, prec)
    nc = _NC_CACHE[key]
    if trace:
        _ensure_ntff_hook()
    from concourse.bass_utils import run_bass_kernel_spmd
    res = run_bass_kernel_spmd(nc, in_maps, list(range(NCORES)), trace=trace)
    out = np.concatenate([res.results[c]["out"] for c in range(NCORES)], 0)
    return np.ascontiguousarray(out, dtype=np.float32), res


def kernel(**inputs) -> np.ndarray:
    out, _ = run(inputs)
    return out

